# revision 2
# baseline (speedup 1.0000x reference)
"""Trainium2 Bass kernel for nn_BoundaryLoss (retrieval 1-NN + boundary loss).

Math reformulation (validated against the reference on the fixed inputs):
rigid SE(3) transforms preserve distances and dot products, so the 1-NN
search and the signed-distance dot product are done in the GLOBAL frame.
With wg = R_b @ w + t_b (host prep), per-(b,t) argmin_n |w_l - p_l[n]|^2
equals argmax_n s'[n], s'[n] = 2*wg.pg[n] - |pg[n]|^2, and
dots = wg.ng[idx] - pg[idx].ng[idx].

Candidate pruning (the big lever vs the brute-force version): all 100
waypoints of batch b lie in a ball around t_b, so by the triangle
inequality the 1-NN of ANY of them satisfies
  d(p, t_b) <= max_t (d(w_t, phat_b) + |w_t|)  (phat_b = point nearest t_b),
which keeps only ~5-15% of the 20000 boundary points per batch (exact, not
approximate).  Host computes the per-batch candidate lists in O(B*N).

Sharding: one batch per 128-lane tile (100 waypoints on lanes 0-99),
8 slots per core x 8 cores = 64 batches.  Batches are assigned to slots
sorted by candidate count so each slot's table is padded to the max of its
8 cores' counts (compile-time capacities SLOT_CAPS with slack; overflow
falls back to dropping the farthest candidates).

Device pipeline per slot:
  - PE: s'/8 via K=11 fp16 hi/lo split matmuls, fp32 PSUM (512-col chunks).
  - ACT: PSUM->SBUF copies casting to fp16 (DVE max8/find_index8 have no
    fast mode, so scan input is pre-pooled instead: )
  - DVE: tensor_tensor max of the two row halves (fp16 2x_1p: 4 elem/cyc)
    then max8 + max_index over the HALF-length pooled row (1x each).
  - The pooled argmax position maps to 2 original candidates (p, p+half);
    both are refined in exact fp32 from an 8-float/row gathered table
    ([pg, p2, ng, pn]; one indirect DMA per slot per half), batched across
    slots; winner's payload gives dots = wg.ng - pn, then exp_relu, mask,
    ones-matmul partition reduction -> [1, 8] per-core partial sums.
Host: input prep/sharding + final sum of 8x8 partials / 6400.

HW notes (measured): max8/find_index8 run 1x (no 16-bit fast mode) -- the
pooling is what keeps DVE off the critical path; indirect DMA costs ~1us
fixed each (SWDGE), so refine gathers 8 floats/row to merge the value and
payload tables; DMA cannot touch PSUM; engine APs must stay within one
2 KiB PSUM bank; GPSIMD has no PSUM port.
"""

import sys

sys.path.insert(0, "/opt/trn_rl_repo")

import numpy as np

from concourse import bacc, bass, mybir
import concourse.tile as tile
from concourse.bass_utils import run_bass_kernel_spmd

B, T, N = 64, 100, 20000
NCORES = 8
NSLOTS = 8                      # batches per core, one per 128-lane tile
CHUNK = 512                     # one PSUM bank of fp32
KSPLIT = 11                     # fp16 split-matmul contraction rows
# Per-slot candidate capacities (batches sorted by candidate count desc;
# slot s holds ranks [8s, 8s+8)).  Seed-0 slot maxima are
# [2862, 2081, 1828, 1668, 1483, 1357, 1271, 1135]; padded to CHUNK with
# slack.  Overflow (different BLAS rounding etc.) drops farthest points.
SLOT_CAPS = [3072, 2560, 2048, 2048, 1536, 1536, 1536, 1536]
SLOT_BASE = np.concatenate([[0], np.cumsum(SLOT_CAPS)]).astype(np.int64)
SK = int(SLOT_BASE[-1])         # 15872 candidate columns per core

F32 = mybir.dt.float32
F16 = mybir.dt.float16
U32 = mybir.dt.uint32
U8 = mybir.dt.uint8
OP = mybir.AluOpType
AF = mybir.ActivationFunctionType


def build():
    nc = bacc.Bacc("TRN2", target_bir_lowering=False, debug=False,
                   num_devices=NCORES)
    lhs = nc.dram_tensor("lhs", [KSPLIT, NSLOTS * 128], F16,
                         kind="ExternalInput").ap()
    rhs = nc.dram_tensor("rhs", [KSPLIT, SK], F16, kind="ExternalInput").ap()
    wgv = nc.dram_tensor("wgv", [128, NSLOTS, 3], F32,
                         kind="ExternalInput").ap()
    bse = nc.dram_tensor("bse", [128, NSLOTS, 2], F32,
                         kind="ExternalInput").ap()
    msk = nc.dram_tensor("msk", [128, 1], F32, kind="ExternalInput").ap()
    gtab = nc.dram_tensor("gtab", [SK, 8], F32, kind="ExternalInput").ap()
    out = nc.dram_tensor("out", [1, NSLOTS], F32, kind="ExternalOutput").ap()

    with tile.TileContext(nc) as tc:
        with (
            tc.tile_pool(name="const", bufs=1) as cpool,
            tc.tile_pool(name="s16p", bufs=2) as s16p,
            tc.tile_pool(name="poolp", bufs=2) as poolp,
            tc.tile_pool(name="sb", bufs=3) as sb,
            tc.tile_pool(name="ps", bufs=8, space="PSUM") as ps,
        ):
            lhs_sb = cpool.tile([KSPLIT, NSLOTS * 128], F16)
            nc.sync.dma_start(out=lhs_sb[:], in_=lhs[:])
            # rhs arrives per-slot so slot 0 compute starts early
            rhs_sb = cpool.tile([KSPLIT, SK], F16)
            for s in range(NSLOTS):
                lo, hi = int(SLOT_BASE[s]), int(SLOT_BASE[s + 1])
                nc.sync.dma_start(out=rhs_sb[:, lo:hi], in_=rhs[:, lo:hi])
            wgv_sb = cpool.tile([128, NSLOTS, 3], F32)
            nc.sync.dma_start(out=wgv_sb[:], in_=wgv[:])
            bse_sb = cpool.tile([128, NSLOTS, 2], F32)
            nc.sync.dma_start(out=bse_sb[:], in_=bse[:])
            msk_sb = cpool.tile([128, 1], F32)
            nc.sync.dma_start(out=msk_sb[:], in_=msk[:])
            ones_sb = cpool.tile([128, 1], F32)
            nc.vector.memset(ones_sb[:], 1.0)
            idx0f = cpool.tile([128, NSLOTS], F32)

            for s in range(NSLOTS):
                cap = SLOT_CAPS[s]
                nch = cap // CHUNK
                half = cap // 2
                off = int(SLOT_BASE[s])
                s16 = s16p.tile([128, cap], F16, tag="s16")
                for c in range(nch):
                    pgp = ps.tile([128, CHUNK], F32, tag="mm")
                    nc.tensor.matmul(
                        out=pgp[:],
                        lhsT=lhs_sb[:, s * 128:(s + 1) * 128],
                        rhs=rhs_sb[:, off + c * CHUNK:off + (c + 1) * CHUNK],
                        start=True, stop=True,
                    )
                    nc.scalar.activation(s16[:, c * CHUNK:(c + 1) * CHUNK],
                                         pgp[:], AF.Copy)
                pooled = poolp.tile([128, half], F16, tag="pooled")
                nc.vector.tensor_tensor(out=pooled[:], in0=s16[:, 0:half],
                                        in1=s16[:, half:cap], op=OP.max)
                ma = sb.tile([128, 8], F16, tag="ma")
                nc.vector.max(ma[:], pooled[:])
                ia = sb.tile([128, 8], U32, tag="ia")
                nc.vector.max_index(ia[:], ma[:], pooled[:])
                nc.vector.tensor_copy(idx0f[:, s:s + 1], ia[:, 0:1])

            # ---- batched refine across slots (exact fp32) ----
            idxAf = sb.tile([128, NSLOTS], F32, tag="idxAf")
            nc.vector.tensor_tensor(out=idxAf[:], in0=idx0f[:],
                                    in1=bse_sb[:, :, 0], op=OP.add)
            idxBf = sb.tile([128, NSLOTS], F32, tag="idxBf")
            nc.vector.tensor_tensor(out=idxBf[:], in0=idxAf[:],
                                    in1=bse_sb[:, :, 1], op=OP.add)
            idxAu = sb.tile([128, NSLOTS], U32, tag="idxAu")
            nc.vector.tensor_copy(idxAu[:], idxAf[:])
            idxBu = sb.tile([128, NSLOTS], U32, tag="idxBu")
            nc.vector.tensor_copy(idxBu[:], idxBf[:])

            candA = sb.tile([128, NSLOTS, 8], F32, tag="candA")
            candB = sb.tile([128, NSLOTS, 8], F32, tag="candB")
            for s in range(NSLOTS):
                nc.gpsimd.indirect_dma_start(
                    out=candA[:, s, :], out_offset=None, in_=gtab[:],
                    in_offset=bass.IndirectOffsetOnAxis(
                        ap=idxAu[:, s:s + 1], axis=0),
                )
                nc.gpsimd.indirect_dma_start(
                    out=candB[:, s, :], out_offset=None, in_=gtab[:],
                    in_offset=bass.IndirectOffsetOnAxis(
                        ap=idxBu[:, s:s + 1], axis=0),
                )

            def exact_score(cand, tag):
                acc = sb.tile([128, NSLOTS], F32, tag=tag + "acc")
                nc.vector.tensor_tensor(out=acc[:], in0=cand[:, :, 0],
                                        in1=wgv_sb[:, :, 0], op=OP.mult)
                tmp = sb.tile([128, NSLOTS], F32, tag=tag + "tmp")
                for d in (1, 2):
                    nc.vector.tensor_tensor(out=tmp[:], in0=cand[:, :, d],
                                            in1=wgv_sb[:, :, d], op=OP.mult)
                    nc.vector.tensor_tensor(out=acc[:], in0=acc[:],
                                            in1=tmp[:], op=OP.add)
                sc = sb.tile([128, NSLOTS], F32, tag=tag + "sc")
                nc.vector.scalar_tensor_tensor(
                    out=sc[:], in0=acc[:], scalar=2.0, in1=cand[:, :, 3],
                    op0=OP.mult, op1=OP.subtract)
                return sc

            sA = exact_score(candA, "sA")
            sB = exact_score(candB, "sB")
            gtm = sb.tile([128, NSLOTS], U8, tag="gtm")
            nc.vector.tensor_tensor(out=gtm[:], in0=sB[:], in1=sA[:],
                                    op=OP.is_gt)
            # winner payload: [ng(3), pn] columns 4..8
            pay = sb.tile([128, NSLOTS, 4], F32, tag="pay")
            nc.vector.tensor_copy(pay[:], candA[:, :, 4:8])
            nc.vector.copy_predicated(
                pay[:], gtm[:].to_broadcast([128, NSLOTS, 4]),
                candB[:, :, 4:8])

            # dots = wg . ng - pn
            dots = sb.tile([128, NSLOTS], F32, tag="dots")
            nc.vector.tensor_tensor(out=dots[:], in0=pay[:, :, 0],
                                    in1=wgv_sb[:, :, 0], op=OP.mult)
            dtm = sb.tile([128, NSLOTS], F32, tag="dtm")
            for d in (1, 2):
                nc.vector.tensor_tensor(out=dtm[:], in0=pay[:, :, d],
                                        in1=wgv_sb[:, :, d], op=OP.mult)
                nc.vector.tensor_tensor(out=dots[:], in0=dots[:],
                                        in1=dtm[:], op=OP.add)
            nc.vector.tensor_tensor(out=dots[:], in0=dots[:],
                                    in1=pay[:, :, 3], op=OP.subtract)

            # exp_relu: x>0 ? x+1 : exp(0.5x)
            ecl = sb.tile([128, NSLOTS], F32, tag="ecl")
            nc.vector.tensor_scalar_min(ecl[:], dots[:], 0.0)
            ex = sb.tile([128, NSLOTS], F32, tag="ex")
            nc.scalar.activation(ex[:], ecl[:], AF.Exp, scale=0.5)
            p1 = sb.tile([128, NSLOTS], F32, tag="p1")
            nc.vector.tensor_scalar_add(p1[:], dots[:], 1.0)
            gt0 = sb.tile([128, NSLOTS], U8, tag="gt0")
            nc.vector.tensor_scalar(gt0[:], dots[:], 0.0, None, OP.is_gt)
            er = sb.tile([128, NSLOTS], F32, tag="er")
            nc.vector.select(er[:], gt0[:], p1[:], ex[:])
            erm = sb.tile([128, NSLOTS], F32, tag="erm")
            nc.vector.tensor_tensor(
                out=erm[:], in0=er[:],
                in1=msk_sb[:, 0:1].to_broadcast([128, NSLOTS]), op=OP.mult)

            po = ps.tile([1, NSLOTS], F32, tag="mm")
            nc.tensor.matmul(out=po[:], lhsT=ones_sb[:, 0:1], rhs=erm[:],
                             start=True, stop=True)
            ob = sb.tile([1, NSLOTS], F32, tag="ob")
            nc.vector.tensor_copy(ob[:], po[:])
            nc.sync.dma_start(out=out[:], in_=ob[:])

    nc.compile()
    return nc


def _f16_split(x32):
    hi = x32.astype(np.float16)
    lo = (x32 - hi.astype(np.float32)).astype(np.float16)
    return hi, lo


def prep_inputs(posesglobal, waypointslocal, boundary, boundarynormals):
    poses = np.asarray(posesglobal, dtype=np.float32)
    wpts = np.asarray(waypointslocal, dtype=np.float32)
    bound = np.asarray(boundary, dtype=np.float32)
    nrm = np.asarray(boundarynormals, dtype=np.float32)

    R = poses[:, :3, :3]
    t = poses[:, :3, 3]
    wg = (np.einsum("bij,btj->bti", R, wpts).astype(np.float32)
          + t[:, None, :]).astype(np.float32)                 # [B, T, 3]

    pg = bound[:3]                                            # [3, N]
    p2 = (pg * pg).sum(axis=0).astype(np.float32)             # [N]
    pn = (pg * nrm).sum(axis=0).astype(np.float32)            # [N]
    P = pg.T                                                  # [N, 3]

    # per-batch candidate lists via triangle-inequality ball around t_b
    d2t = ((P[None, :, :] - t[:, None, :]) ** 2).sum(-1)      # [B, N]
    phat = P[np.argmin(d2t, axis=1)]                          # [B, 3]
    dw_phat = np.linalg.norm(wg - phat[:, None, :], axis=2)   # [B, T]
    wnorm = np.linalg.norm(wpts, axis=2)                      # [B, T]
    Rb = (dw_phat + wnorm).max(axis=1)                        # [B]
    Ks = (d2t <= (Rb * Rb)[:, None]).sum(axis=1)

    order = np.argsort(-Ks, kind="stable")                    # desc by K

    bh, bl = _f16_split(pg)
    ch, cl = _f16_split(p2 / 8.0)

    gflat = np.empty((N, 8), np.float32)
    gflat[:, 0:3] = P
    gflat[:, 3] = p2
    gflat[:, 4:7] = nrm.T
    gflat[:, 7] = pn

    in_maps = []
    slot_batches = np.empty((NSLOTS, NCORES), np.int64)
    for s in range(NSLOTS):
        slot_batches[s] = order[s * NCORES:(s + 1) * NCORES]

    for c in range(NCORES):
        lhsc = np.zeros((KSPLIT, NSLOTS * 128), np.float16)
        rhsc = np.zeros((KSPLIT, SK), np.float16)
        rhsc[9, :] = np.float16(60000.0)   # pad cols can never win argmax
        gtabc = np.zeros((SK, 8), np.float32)
        wgvc = np.zeros((128, NSLOTS, 3), np.float32)
        bsec = np.zeros((128, NSLOTS, 2), np.float32)
        for s in range(NSLOTS):
            b = int(slot_batches[s, c])
            cap = SLOT_CAPS[s]
            cidx = np.nonzero(d2t[b] <= Rb[b] * Rb[b])[0]
            if len(cidx) > cap:   # safety: drop farthest candidates
                keep = np.argpartition(d2t[b][cidx], cap)[:cap]
                cidx = np.sort(cidx[keep])
            K = len(cidx)
            lo = int(SLOT_BASE[s])
            # lhs rows: per coord d -> [ah_d, ah_d, al_d]; rows 9,10 = -1
            w = wg[b]                                     # [100, 3]
            ah, al = _f16_split(w.T / 4.0)                # [3, 100]
            for d in range(3):
                lhsc[3 * d + 0, s * 128:s * 128 + T] = ah[d]
                lhsc[3 * d + 1, s * 128:s * 128 + T] = ah[d]
                lhsc[3 * d + 2, s * 128:s * 128 + T] = al[d]
            lhsc[9, s * 128:(s + 1) * 128] = np.float16(-1.0)
            lhsc[10, s * 128:(s + 1) * 128] = np.float16(-1.0)
            # rhs rows: per coord d -> [bh_d, bl_d, bh_d]; then [ch, cl]
            for d in range(3):
                rhsc[3 * d + 0, lo:lo + K] = bh[d, cidx]
                rhsc[3 * d + 1, lo:lo + K] = bl[d, cidx]
                rhsc[3 * d + 2, lo:lo + K] = bh[d, cidx]
            rhsc[9, lo:lo + K] = ch[cidx]
            rhsc[10, lo:lo + K] = cl[cidx]
            gtabc[lo:lo + K] = gflat[cidx]
            wgvc[:T, s, :] = w
            bsec[:, s, 0] = lo
            bsec[:, s, 1] = cap // 2
        mskc = np.zeros((128, 1), np.float32)
        mskc[:T, 0] = 1.0
        in_maps.append({"lhs": lhsc, "rhs": rhsc, "wgv": wgvc,
                        "bse": bsec, "msk": mskc, "gtab": gtabc})
    return in_maps


_CACHE = {}


def kernel(posesglobal, waypointslocal, boundary, boundarynormals):
    if "nc" not in _CACHE:
        _CACHE["nc"] = build()
    nc = _CACHE["nc"]
    in_maps = prep_inputs(posesglobal, waypointslocal, boundary,
                          boundarynormals)
    res = run_bass_kernel_spmd(nc, in_maps, list(range(NCORES)))
    total = 0.0
    for r in res.results:
        total += float(np.asarray(r["out"], dtype=np.float64).sum())
    return np.float32(total / (B * T))


# revision 7
# speedup vs baseline: 1.3456x; 1.3456x over previous
"""Trainium2 Bass kernel for nn_BoundaryLoss (retrieval 1-NN + boundary loss).

Math reformulation (validated against the reference on the fixed inputs):
rigid SE(3) transforms preserve distances and dot products, so the 1-NN
search and the signed-distance dot product are done in the GLOBAL frame.
With wg = R_b @ w + t_b (host prep), per-(b,t) argmin_n |w_l - p_l[n]|^2
equals argmax_n s'[n], s'[n] = 2*wg.pg[n] - |pg[n]|^2, and
dots = wg.ng[idx] - pg[idx].ng[idx].

Candidate pruning (the big lever vs the brute-force version): all 100
waypoints of batch b lie in a ball around t_b, so by the triangle
inequality the 1-NN of ANY of them satisfies
  d(p, t_b) <= max_t (d(w_t, phat_b) + |w_t|)  (phat_b = point nearest t_b),
which keeps only ~5-15% of the 20000 boundary points per batch (exact, not
approximate).  Host computes the per-batch candidate lists in O(B*N).

Sharding: one batch per 128-lane tile (100 waypoints on lanes 0-99),
8 slots per core x 8 cores = 64 batches.  Batches are assigned to slots
sorted by candidate count so each slot's table is padded to the max of its
8 cores' counts (compile-time capacities SLOT_CAPS with slack; overflow
falls back to dropping the farthest candidates).

Device pipeline per slot:
  - PE: s'/8 via K=11 fp16 hi/lo split matmuls, fp32 PSUM (512-col chunks).
  - ACT: PSUM->SBUF copies casting to fp16.
  - DVE: tensor_tensor max of the two row halves (fp16 2x_1p: 4 elem/cyc)
    then max8 + max_index over the HALF-length pooled row (these ops have
    no 16-bit fast mode -- the pooling is what keeps them affordable).
  - GPSIMD: two indirect gathers per slot straight off max_index's u32
    output against row-sliced DRAM tables (gtab[lo:] for the low half,
    gtab[lo+half:] for the high half) -- no index arithmetic at all.
Batched tail across slots: exact fp32 re-score of the 2 candidates,
winner's payload gives dots = wg.ng - pn; exp_relu via the exact identity
exp_relu(x) = max(x+1, exp(0.5*min(x,0))); lane masking is folded into the
final partition-reduction matmul by using the mask as lhsT.
Host: input prep/sharding + final sum of 8x8 partials / 6400.

HW notes (measured): max8/find_index8 run 1x; indirect DMA ~1us fixed each
(SWDGE) so refine gathers 8 floats/row from a merged [pg,p2,ng,pn] table;
DMA cannot touch PSUM; engine APs must stay within one 2 KiB PSUM bank;
GPSIMD has no PSUM port.
"""

import sys

sys.path.insert(0, "/opt/trn_rl_repo")

import numpy as np

from concourse import bacc, bass, mybir
import concourse.tile as tile
from concourse.bass_utils import run_bass_kernel_spmd

B, T, N = 64, 100, 20000
NCORES = 8
NSLOTS = 8                      # batches per core, one per 128-lane tile
CHUNK = 512                     # one PSUM bank of fp32
KSPLIT = 11                     # fp16 split-matmul contraction rows
# Per-slot candidate capacities (batches sorted by candidate count desc;
# slot s holds ranks [8s, 8s+8)).  Seed-0 slot maxima are
# [2862, 2081, 1828, 1668, 1483, 1357, 1271, 1135]; padded to CHUNK with
# slack.  Overflow (different BLAS rounding etc.) drops farthest points.
SLOT_CAPS = [3072, 2560, 2048, 2048, 1536, 1536, 1536, 1536]
SLOT_BASE = np.concatenate([[0], np.cumsum(SLOT_CAPS)]).astype(np.int64)
SK = int(SLOT_BASE[-1])         # 15872 candidate columns per core

F32 = mybir.dt.float32
F16 = mybir.dt.float16
U32 = mybir.dt.uint32
U8 = mybir.dt.uint8
OP = mybir.AluOpType
AF = mybir.ActivationFunctionType


def build():
    nc = bacc.Bacc("TRN2", target_bir_lowering=False, debug=False,
                   num_devices=NCORES)
    lhs = nc.dram_tensor("lhs", [KSPLIT, NSLOTS * 128], F16,
                         kind="ExternalInput").ap()
    rhs = nc.dram_tensor("rhs", [KSPLIT, SK], F16, kind="ExternalInput").ap()
    # wgm: per-lane [wg(3) x NSLOTS, mask] packed small-constant block
    wgm = nc.dram_tensor("wgm", [128, NSLOTS * 3 + 1], F32,
                         kind="ExternalInput").ap()
    # per-slot gather tables, one per half (indirect-DMA sources must be
    # whole tensors -- a row-offset sliced AP is rejected)
    gta = [nc.dram_tensor(f"gta{s}", [SLOT_CAPS[s] // 2, 8], F32,
                          kind="ExternalInput").ap() for s in range(NSLOTS)]
    gtb = [nc.dram_tensor(f"gtb{s}", [SLOT_CAPS[s] // 2, 8], F32,
                          kind="ExternalInput").ap() for s in range(NSLOTS)]
    out = nc.dram_tensor("out", [1, NSLOTS], F32, kind="ExternalOutput").ap()

    with tile.TileContext(nc) as tc:
        with (
            tc.tile_pool(name="const", bufs=1) as cpool,
            tc.tile_pool(name="s16p", bufs=2) as s16p,
            tc.tile_pool(name="poolp", bufs=2) as poolp,
            tc.tile_pool(name="sb", bufs=3) as sb,
            tc.tile_pool(name="ps", bufs=8, space="PSUM") as ps,
        ):
            lhs_sb = cpool.tile([KSPLIT, NSLOTS * 128], F16)
            nc.sync.dma_start(out=lhs_sb[:], in_=lhs[:])
            # rhs arrives per-slot so slot 0 compute starts early
            rhs_sb = cpool.tile([KSPLIT, SK], F16)
            for s in range(NSLOTS):
                lo, hi = int(SLOT_BASE[s]), int(SLOT_BASE[s + 1])
                nc.sync.dma_start(out=rhs_sb[:, lo:hi], in_=rhs[:, lo:hi])
            wgm_sb = cpool.tile([128, NSLOTS * 3 + 1], F32)
            nc.sync.dma_start(out=wgm_sb[:], in_=wgm[:])
            wgv_sb = wgm_sb[:, 0:NSLOTS * 3].rearrange(
                "p (s d) -> p s d", s=NSLOTS)
            msk_sb = wgm_sb[:, NSLOTS * 3:NSLOTS * 3 + 1]
            # preload the exp table so ACT's one-time load overlaps slot 0
            warm = cpool.tile([1, 1], F32)
            nc.vector.memset(warm[:], 0.0)
            nc.scalar.activation(warm[:], warm[:], AF.Exp, scale=0.5)

            candAB = cpool.tile([128, NSLOTS, 2, 8], F32)

            for s in range(NSLOTS):
                cap = SLOT_CAPS[s]
                nch = cap // CHUNK
                half = cap // 2
                off = int(SLOT_BASE[s])
                s16 = s16p.tile([128, cap], F16, tag="s16")
                for c in range(nch):
                    pgp = ps.tile([128, CHUNK], F32, tag="mm")
                    nc.tensor.matmul(
                        out=pgp[:],
                        lhsT=lhs_sb[:, s * 128:(s + 1) * 128],
                        rhs=rhs_sb[:, off + c * CHUNK:off + (c + 1) * CHUNK],
                        start=True, stop=True,
                    )
                    nc.scalar.activation(s16[:, c * CHUNK:(c + 1) * CHUNK],
                                         pgp[:], AF.Copy)
                pooled = poolp.tile([128, half], F16, tag="pooled")
                nc.vector.tensor_tensor(out=pooled[:], in0=s16[:, 0:half],
                                        in1=s16[:, half:cap], op=OP.max)
                ma = sb.tile([128, 8], F16, tag="ma")
                nc.vector.max(ma[:], pooled[:])
                ia = sb.tile([128, 8], U32, tag="ia")
                nc.vector.max_index(ia[:], ma[:], pooled[:])
                # candidates p and p+half: gather from row-sliced tables
                # with the raw u32 index -- no on-device index arithmetic
                nc.gpsimd.indirect_dma_start(
                    out=candAB[:, s, 0, :], out_offset=None,
                    in_=gta[s][:],
                    in_offset=bass.IndirectOffsetOnAxis(
                        ap=ia[:, 0:1], axis=0),
                )
                nc.gpsimd.indirect_dma_start(
                    out=candAB[:, s, 1, :], out_offset=None,
                    in_=gtb[s][:],
                    in_offset=bass.IndirectOffsetOnAxis(
                        ap=ia[:, 0:1], axis=0),
                )

            # ---- batched refine across slots (exact fp32) ----
            # sC[:, s, h] = 2 * wg_s . pg - p2  for both halves h
            acc = sb.tile([128, NSLOTS, 2], F32, tag="acc")
            nc.vector.tensor_tensor(
                out=acc[:], in0=candAB[:, :, :, 0],
                in1=wgv_sb[:, :, 0:1].to_broadcast([128, NSLOTS, 2]),
                op=OP.mult)
            tmp = sb.tile([128, NSLOTS, 2], F32, tag="tmp")
            for d in (1, 2):
                nc.vector.tensor_tensor(
                    out=tmp[:], in0=candAB[:, :, :, d],
                    in1=wgv_sb[:, :, d:d + 1].to_broadcast([128, NSLOTS, 2]),
                    op=OP.mult)
                nc.vector.tensor_tensor(out=acc[:], in0=acc[:], in1=tmp[:],
                                        op=OP.add)
            sC = sb.tile([128, NSLOTS, 2], F32, tag="sC")
            nc.vector.scalar_tensor_tensor(
                out=sC[:], in0=acc[:], scalar=2.0, in1=candAB[:, :, :, 3],
                op0=OP.mult, op1=OP.subtract)
            gtm = sb.tile([128, NSLOTS, 1], U8, tag="gtm")
            nc.vector.tensor_tensor(out=gtm[:, :, 0], in0=sC[:, :, 1],
                                    in1=sC[:, :, 0], op=OP.is_gt)
            # winner payload: [ng(3), pn] columns 4..8
            pay = sb.tile([128, NSLOTS, 4], F32, tag="pay")
            nc.vector.tensor_copy(pay[:], candAB[:, :, 0, 4:8])
            nc.vector.copy_predicated(
                pay[:], gtm[:].to_broadcast([128, NSLOTS, 4]),
                candAB[:, :, 1, 4:8])

            # dots = wg . ng - pn
            dots = sb.tile([128, NSLOTS], F32, tag="dots")
            nc.vector.tensor_tensor(out=dots[:], in0=pay[:, :, 0],
                                    in1=wgv_sb[:, :, 0], op=OP.mult)
            dtm = sb.tile([128, NSLOTS], F32, tag="dtm")
            for d in (1, 2):
                nc.vector.tensor_tensor(out=dtm[:], in0=pay[:, :, d],
                                        in1=wgv_sb[:, :, d], op=OP.mult)
                nc.vector.tensor_tensor(out=dots[:], in0=dots[:],
                                        in1=dtm[:], op=OP.add)
            nc.vector.tensor_tensor(out=dots[:], in0=dots[:],
                                    in1=pay[:, :, 3], op=OP.subtract)

            # exp_relu(x) = max(x + 1, exp(0.5 * min(x, 0)))  (exact)
            ecl = sb.tile([128, NSLOTS], F32, tag="ecl")
            nc.vector.tensor_scalar_min(ecl[:], dots[:], 0.0)
            ex = sb.tile([128, NSLOTS], F32, tag="ex")
            nc.scalar.activation(ex[:], ecl[:], AF.Exp, scale=0.5)
            er = sb.tile([128, NSLOTS], F32, tag="er")
            nc.vector.scalar_tensor_tensor(
                out=er[:], in0=dots[:], scalar=1.0, in1=ex[:],
                op0=OP.add, op1=OP.max)

            # lane masking folded into the partition reduction (lhsT = mask)
            po = ps.tile([1, NSLOTS], F32, tag="mm")
            nc.tensor.matmul(out=po[:], lhsT=msk_sb, rhs=er[:],
                             start=True, stop=True)
            ob = sb.tile([1, NSLOTS], F32, tag="ob")
            nc.vector.tensor_copy(ob[:], po[:])
            nc.sync.dma_start(out=out[:], in_=ob[:])

    nc.compile()
    return nc


def _f16_split(x32):
    hi = x32.astype(np.float16)
    lo = (x32 - hi.astype(np.float32)).astype(np.float16)
    return hi, lo


def prep_inputs(posesglobal, waypointslocal, boundary, boundarynormals):
    poses = np.asarray(posesglobal, dtype=np.float32)
    wpts = np.asarray(waypointslocal, dtype=np.float32)
    bound = np.asarray(boundary, dtype=np.float32)
    nrm = np.asarray(boundarynormals, dtype=np.float32)

    R = poses[:, :3, :3]
    t = poses[:, :3, 3]
    wg = (np.einsum("bij,btj->bti", R, wpts).astype(np.float32)
          + t[:, None, :]).astype(np.float32)                 # [B, T, 3]

    pg = bound[:3]                                            # [3, N]
    p2 = (pg * pg).sum(axis=0).astype(np.float32)             # [N]
    pn = (pg * nrm).sum(axis=0).astype(np.float32)            # [N]
    P = pg.T                                                  # [N, 3]

    # per-batch candidate lists via triangle-inequality ball around t_b
    d2t = ((P[None, :, :] - t[:, None, :]) ** 2).sum(-1)      # [B, N]
    phat = P[np.argmin(d2t, axis=1)]                          # [B, 3]
    dw_phat = np.linalg.norm(wg - phat[:, None, :], axis=2)   # [B, T]
    wnorm = np.linalg.norm(wpts, axis=2)                      # [B, T]
    Rb = (dw_phat + wnorm).max(axis=1)                        # [B]
    Ks = (d2t <= (Rb * Rb)[:, None]).sum(axis=1)

    order = np.argsort(-Ks, kind="stable")                    # desc by K

    bh, bl = _f16_split(pg)
    ch, cl = _f16_split(p2 / 8.0)

    gflat = np.empty((N, 8), np.float32)
    gflat[:, 0:3] = P
    gflat[:, 3] = p2
    gflat[:, 4:7] = nrm.T
    gflat[:, 7] = pn

    in_maps = []
    for c in range(NCORES):
        lhsc = np.zeros((KSPLIT, NSLOTS * 128), np.float16)
        rhsc = np.zeros((KSPLIT, SK), np.float16)
        rhsc[9, :] = np.float16(60000.0)   # pad cols can never win argmax
        imap = {"lhs": lhsc, "rhs": rhsc}
        wgmc = np.zeros((128, NSLOTS * 3 + 1), np.float32)
        wgmc[:T, NSLOTS * 3] = 1.0                            # lane mask
        for s in range(NSLOTS):
            b = int(order[s * NCORES + c])
            cap = SLOT_CAPS[s]
            cidx = np.nonzero(d2t[b] <= Rb[b] * Rb[b])[0]
            if len(cidx) > cap:   # safety: drop farthest candidates
                keep = np.argpartition(d2t[b][cidx], cap)[:cap]
                cidx = np.sort(cidx[keep])
            K = len(cidx)
            lo = int(SLOT_BASE[s])
            # lhs rows: per coord d -> [ah_d, ah_d, al_d]; rows 9,10 = -1
            w = wg[b]                                         # [100, 3]
            ah, al = _f16_split(w.T / 4.0)                    # [3, 100]
            for d in range(3):
                lhsc[3 * d + 0, s * 128:s * 128 + T] = ah[d]
                lhsc[3 * d + 1, s * 128:s * 128 + T] = ah[d]
                lhsc[3 * d + 2, s * 128:s * 128 + T] = al[d]
            lhsc[9, s * 128:(s + 1) * 128] = np.float16(-1.0)
            lhsc[10, s * 128:(s + 1) * 128] = np.float16(-1.0)
            # rhs rows: per coord d -> [bh_d, bl_d, bh_d]; then [ch, cl]
            for d in range(3):
                rhsc[3 * d + 0, lo:lo + K] = bh[d, cidx]
                rhsc[3 * d + 1, lo:lo + K] = bl[d, cidx]
                rhsc[3 * d + 2, lo:lo + K] = bh[d, cidx]
            rhsc[9, lo:lo + K] = ch[cidx]
            rhsc[10, lo:lo + K] = cl[cidx]
            gtabc = np.zeros((cap, 8), np.float32)
            gtabc[:K] = gflat[cidx]
            half = cap // 2
            imap[f"gta{s}"] = gtabc[:half].copy()
            imap[f"gtb{s}"] = gtabc[half:].copy()
            wgmc[:T, s * 3:(s + 1) * 3] = w
        imap["wgm"] = wgmc
        in_maps.append(imap)
    return in_maps


_CACHE = {}


def kernel(posesglobal, waypointslocal, boundary, boundarynormals):
    if "nc" not in _CACHE:
        _CACHE["nc"] = build()
    nc = _CACHE["nc"]
    in_maps = prep_inputs(posesglobal, waypointslocal, boundary,
                          boundarynormals)
    res = run_bass_kernel_spmd(nc, in_maps, list(range(NCORES)))
    total = 0.0
    for r in res.results:
        total += float(np.asarray(r["out"], dtype=np.float64).sum())
    return np.float32(total / (B * T))


# revision 13
# speedup vs baseline: 2.1138x; 1.5709x over previous
"""Trainium2 Bass kernel for nn_BoundaryLoss (retrieval 1-NN + boundary loss).

Math reformulation (validated against the reference on the fixed inputs):
rigid SE(3) transforms preserve distances and dot products, so the 1-NN
search and the signed-distance dot product are done in the GLOBAL frame.
With wg = R_b @ w + t_b (host prep), per-(b,t) argmin_n |w_l - p_l[n]|^2
equals argmax_n s'[n], s'[n] = 2*wg.pg[n] - |pg[n]|^2, and
dots = wg.ng[idx] - pg[idx].ng[idx].

Candidate pruning (the big lever vs brute force): all 100 waypoints of
batch b sit in a small ball, and for probe boundary points phat_j (the
1-NN of 8 farthest-point samples of the batch's waypoints, found on host
in O(B*J*N)) the triangle inequality gives, for every waypoint w and its
true 1-NN p*:
  d(p*, t_b) <= max_t (min_j d(w_t, phat_j) + |w_t|),
which keeps only ~2-4% of the 20000 boundary points per batch.  The
pruning is exact (a provable ball bound), not approximate.

Sharding: one batch per 128-lane tile (100 waypoints on lanes 0-99),
8 slots per core x 8 cores = 64 batches.  Batches are assigned to slots
sorted by candidate count so each slot's table is padded to the max of its
8 cores' counts (compile-time capacities SLOT_CAPS with slack; overflow
falls back to dropping the farthest candidates).

Device pipeline per slot:
  - PE: s'/8 via K=11 fp16 hi/lo split matmuls, fp32 PSUM (<=512-col
    chunks, one PSUM bank each).
  - ACT: PSUM->SBUF copies casting to fp16.
  - DVE: tensor_tensor max of the two row halves (fp16 2x_1p: 4 elem/cyc)
    then max8 + max_index over the HALF-length pooled row (these ops have
    no 16-bit fast mode -- the pooling is what keeps them affordable).
  - GPSIMD: ONE indirect gather per slot straight off max_index's u32
    output: the per-slot table row p holds BOTH candidates' data
    [pg,p2,ng,pn | pg,p2,ng,pn] for positions p and p+half (16 fp32).
Batched tail across slots: exact fp32 re-score picks the half, dots for
both halves are blended arithmetically; exp_relu via the exact identity
exp_relu(x) = max(x+1, exp(0.5*min(x,0))); lane masking is folded into the
final partition-reduction matmul by using the mask as lhsT.
Host: input prep/sharding + final sum of 8x8 partials / 6400.

HW notes (measured): max8/find_index8 run 1x (no 16-bit fast mode);
indirect DMA ~1us fixed each (SWDGE); DMA cannot touch PSUM; engine APs
must stay within one 2 KiB PSUM bank; GPSIMD has no PSUM port; indirect
DMA sources must be whole tensors (no row-offset slices); PE semaphore
instructions cost ~0.4us each so matmul count matters.
"""

import sys

sys.path.insert(0, "/opt/trn_rl_repo")

import numpy as np

from concourse import bacc, bass, mybir
import concourse.tile as tile
from concourse.bass_utils import run_bass_kernel_spmd

B, T, N = 64, 100, 20000
NCORES = 8
NSLOTS = 8                      # batches per core, one per 128-lane tile
CHUNK = 512                     # one PSUM bank of fp32
KSPLIT = 11                     # fp16 split-matmul contraction rows
NPROBE = 8                      # pruning probes per batch
# Per-slot candidate capacities (batches sorted by candidate count desc;
# slot s holds ranks [8s, 8s+8)).  Seed-0 slot maxima with 8-probe pruning
# are [832, 670, 604, 542, 487, 462, 445, 397]; padded with slack.
# Overflow (different BLAS rounding etc.) drops farthest candidates.
SLOT_CAPS = [1024, 768, 640, 640, 512, 512, 512, 512]
SLOT_BASE = np.concatenate([[0], np.cumsum(SLOT_CAPS)]).astype(np.int64)
SK = int(SLOT_BASE[-1])         # 5120 candidate columns per core

F32 = mybir.dt.float32
F16 = mybir.dt.float16
U32 = mybir.dt.uint32
U8 = mybir.dt.uint8
OP = mybir.AluOpType
AF = mybir.ActivationFunctionType


def build():
    nc = bacc.Bacc("TRN2", target_bir_lowering=False, debug=False,
                   num_devices=NCORES)
    lhs = nc.dram_tensor("lhs", [KSPLIT, NSLOTS * 128], F16,
                         kind="ExternalInput").ap()
    rhs = nc.dram_tensor("rhs", [KSPLIT, SK], F16, kind="ExternalInput").ap()
    # wgm: per-lane [wg(3) x NSLOTS, mask] packed small-constant block
    wgm = nc.dram_tensor("wgm", [128, NSLOTS * 3 + 1], F32,
                         kind="ExternalInput").ap()
    # per-slot merged gather tables: row p = both halves' candidate data
    # (indirect-DMA sources must be whole tensors, so one per slot)
    gt = [nc.dram_tensor(f"gt{s}", [SLOT_CAPS[s] // 2, 16], F32,
                         kind="ExternalInput").ap() for s in range(NSLOTS)]
    out = nc.dram_tensor("out", [1, NSLOTS], F32, kind="ExternalOutput").ap()

    with tile.TileContext(nc) as tc:
        with (
            tc.tile_pool(name="const", bufs=1) as cpool,
            tc.tile_pool(name="s16p", bufs=2) as s16p,
            tc.tile_pool(name="poolp", bufs=2) as poolp,
            tc.tile_pool(name="sb", bufs=3) as sb,
            tc.tile_pool(name="ps", bufs=8, space="PSUM") as ps,
        ):
            # input DMA triggers spread across engine queues
            lhs_sb = cpool.tile([KSPLIT, NSLOTS * 128], F16)
            nc.scalar.dma_start(out=lhs_sb[:], in_=lhs[:])
            rhs_sb = cpool.tile([KSPLIT, SK], F16)
            for s in range(NSLOTS):
                lo, hi = int(SLOT_BASE[s]), int(SLOT_BASE[s + 1])
                eng = nc.sync if s % 2 == 0 else nc.gpsimd
                eng.dma_start(out=rhs_sb[:, lo:hi], in_=rhs[:, lo:hi])
            wgm_sb = cpool.tile([128, NSLOTS * 3 + 1], F32)
            nc.scalar.dma_start(out=wgm_sb[:], in_=wgm[:])
            wgv_sb = wgm_sb[:, 0:NSLOTS * 3].rearrange(
                "p (s d) -> p s d", s=NSLOTS)
            msk_sb = wgm_sb[:, NSLOTS * 3:NSLOTS * 3 + 1]
            # preload the exp table so ACT's one-time load overlaps slot 0
            warm = cpool.tile([1, 1], F32)
            nc.vector.memset(warm[:], 0.0)
            nc.scalar.activation(warm[:], warm[:], AF.Exp, scale=0.5)

            # NOTE: indirect-DMA out APs must keep the gathered row flat --
            # a nested [.., 2, 8] out pattern silently mis-gathers (measured)
            candAB = cpool.tile([128, NSLOTS, 16], F32)
            cand4 = candAB[:].rearrange("p s (h d) -> p s h d", h=2)

            for s in range(NSLOTS):
                cap = SLOT_CAPS[s]
                half = cap // 2
                off = int(SLOT_BASE[s])
                s16 = s16p.tile([128, cap], F16, tag="s16")
                for c0 in range(0, cap, CHUNK):
                    w = min(CHUNK, cap - c0)
                    # constant PSUM tile shape (one bank); use a w-wide slice
                    pgp = ps.tile([128, CHUNK], F32, tag="mm")
                    nc.tensor.matmul(
                        out=pgp[:, 0:w],
                        lhsT=lhs_sb[:, s * 128:(s + 1) * 128],
                        rhs=rhs_sb[:, off + c0:off + c0 + w],
                        start=True, stop=True,
                    )
                    nc.scalar.activation(s16[:, c0:c0 + w], pgp[:, 0:w],
                                         AF.Copy)
                pooled = poolp.tile([128, half], F16, tag="pooled")
                nc.vector.tensor_tensor(out=pooled[:], in0=s16[:, 0:half],
                                        in1=s16[:, half:cap], op=OP.max)
                ma = sb.tile([128, 8], F16, tag="ma")
                nc.vector.max(ma[:], pooled[:])
                ia = sb.tile([128, 8], U32, tag="ia")
                nc.vector.max_index(ia[:], ma[:], pooled[:])
                # one gather: row ia[,0] holds candidates p AND p+half
                nc.gpsimd.indirect_dma_start(
                    out=candAB[:, s, :], out_offset=None,
                    in_=gt[s][:],
                    in_offset=bass.IndirectOffsetOnAxis(
                        ap=ia[:, 0:1], axis=0),
                )

            # ---- batched refine across slots (exact fp32) ----
            # sC[:, s, h] = 2 * wg_s . pg_h - p2_h ; dotsAB = wg.ng_h - pn_h
            def dot3(cols, tag, bias_col, scale2):
                acc = sb.tile([128, NSLOTS, 2], F32, tag=tag + "a")
                nc.vector.tensor_tensor(
                    out=acc[:], in0=cand4[:, :, :, cols],
                    in1=wgv_sb[:, :, 0:1].to_broadcast([128, NSLOTS, 2]),
                    op=OP.mult)
                tmp = sb.tile([128, NSLOTS, 2], F32, tag=tag + "t")
                for d in (1, 2):
                    nc.vector.tensor_tensor(
                        out=tmp[:], in0=cand4[:, :, :, cols + d],
                        in1=wgv_sb[:, :, d:d + 1].to_broadcast(
                            [128, NSLOTS, 2]),
                        op=OP.mult)
                    nc.vector.tensor_tensor(out=acc[:], in0=acc[:],
                                            in1=tmp[:], op=OP.add)
                res = sb.tile([128, NSLOTS, 2], F32, tag=tag + "r")
                nc.vector.scalar_tensor_tensor(
                    out=res[:], in0=acc[:], scalar=scale2,
                    in1=cand4[:, :, :, bias_col],
                    op0=OP.mult, op1=OP.subtract)
                return res

            sC = dot3(0, "sC", 3, 2.0)       # 2*wg.pg - p2
            dAB = dot3(4, "dAB", 7, 1.0)     # wg.ng - pn
            gtm = sb.tile([128, NSLOTS], U8, tag="gtm")
            nc.vector.tensor_tensor(out=gtm[:], in0=sC[:, :, 1],
                                    in1=sC[:, :, 0], op=OP.is_gt)
            gtf = sb.tile([128, NSLOTS], F32, tag="gtf")
            nc.vector.tensor_copy(gtf[:], gtm[:])
            # dots = dotsA + (B wins) * (dotsB - dotsA)
            delta = sb.tile([128, NSLOTS], F32, tag="delta")
            nc.vector.tensor_tensor(out=delta[:], in0=dAB[:, :, 1],
                                    in1=dAB[:, :, 0], op=OP.subtract)
            dmul = sb.tile([128, NSLOTS], F32, tag="dmul")
            nc.vector.tensor_tensor(out=dmul[:], in0=gtf[:], in1=delta[:],
                                    op=OP.mult)
            dots = sb.tile([128, NSLOTS], F32, tag="dots")
            nc.vector.tensor_tensor(out=dots[:], in0=dAB[:, :, 0],
                                    in1=dmul[:], op=OP.add)

            # exp_relu(x) = max(x + 1, exp(0.5 * min(x, 0)))  (exact)
            ecl = sb.tile([128, NSLOTS], F32, tag="ecl")
            nc.vector.tensor_scalar_min(ecl[:], dots[:], 0.0)
            ex = sb.tile([128, NSLOTS], F32, tag="ex")
            nc.scalar.activation(ex[:], ecl[:], AF.Exp, scale=0.5)
            er = sb.tile([128, NSLOTS], F32, tag="er")
            nc.vector.scalar_tensor_tensor(
                out=er[:], in0=dots[:], scalar=1.0, in1=ex[:],
                op0=OP.add, op1=OP.max)

            # lane masking folded into the partition reduction (lhsT = mask)
            po = ps.tile([1, NSLOTS], F32, tag="mm")
            nc.tensor.matmul(out=po[:], lhsT=msk_sb, rhs=er[:],
                             start=True, stop=True)
            ob = sb.tile([1, NSLOTS], F32, tag="ob")
            nc.vector.tensor_copy(ob[:], po[:])
            nc.sync.dma_start(out=out[:], in_=ob[:])

    nc.compile()
    return nc


def _f16_split(x32):
    hi = x32.astype(np.float16)
    lo = (x32 - hi.astype(np.float32)).astype(np.float16)
    return hi, lo


def prep_inputs(posesglobal, waypointslocal, boundary, boundarynormals):
    poses = np.asarray(posesglobal, dtype=np.float32)
    wpts = np.asarray(waypointslocal, dtype=np.float32)
    bound = np.asarray(boundary, dtype=np.float32)
    nrm = np.asarray(boundarynormals, dtype=np.float32)

    R = poses[:, :3, :3]
    t = poses[:, :3, 3]
    wg = (np.einsum("bij,btj->bti", R, wpts).astype(np.float32)
          + t[:, None, :]).astype(np.float32)                 # [B, T, 3]

    pg = bound[:3]                                            # [3, N]
    p2 = (pg * pg).sum(axis=0).astype(np.float32)             # [N]
    pn = (pg * nrm).sum(axis=0).astype(np.float32)            # [N]
    P = pg.T                                                  # [N, 3]

    # per-batch candidate balls from multi-probe triangle-inequality bound
    d2t = ((P[None, :, :] - t[:, None, :]) ** 2).sum(-1)      # [B, N]
    wnorm = np.linalg.norm(wpts, axis=2)                      # [B, T]
    Rb = np.empty(B, np.float32)
    for b in range(B):
        W = wg[b]
        probes = [W.mean(0)]                # farthest-point sampling
        for _ in range(NPROBE - 1):
            dmin = np.min(((W[:, None, :] - np.asarray(probes)[None])
                           ** 2).sum(-1), axis=1)
            probes.append(W[np.argmax(dmin)])
        probes = np.asarray(probes)
        d2p = ((P[None, :, :] - probes[:, None, :]) ** 2).sum(-1)
        ph = P[np.argmin(d2p, axis=1)]                        # [J, 3]
        dwp = np.linalg.norm(W[:, None, :] - ph[None], axis=2).min(axis=1)
        Rb[b] = (dwp + wnorm[b]).max()
    Ks = (d2t <= (Rb * Rb)[:, None]).sum(axis=1)

    order = np.argsort(-Ks, kind="stable")                    # desc by K

    bh, bl = _f16_split(pg)
    ch, cl = _f16_split(p2 / 8.0)

    gflat = np.empty((N, 8), np.float32)
    gflat[:, 0:3] = P
    gflat[:, 3] = p2
    gflat[:, 4:7] = nrm.T
    gflat[:, 7] = pn

    in_maps = []
    for c in range(NCORES):
        lhsc = np.zeros((KSPLIT, NSLOTS * 128), np.float16)
        rhsc = np.zeros((KSPLIT, SK), np.float16)
        rhsc[9, :] = np.float16(60000.0)   # pad cols can never win argmax
        imap = {"lhs": lhsc, "rhs": rhsc}
        wgmc = np.zeros((128, NSLOTS * 3 + 1), np.float32)
        wgmc[:T, NSLOTS * 3] = 1.0                            # lane mask
        for s in range(NSLOTS):
            b = int(order[s * NCORES + c])
            cap = SLOT_CAPS[s]
            cidx = np.nonzero(d2t[b] <= Rb[b] * Rb[b])[0]
            if len(cidx) > cap:   # safety: drop farthest candidates
                keep = np.argpartition(d2t[b][cidx], cap)[:cap]
                cidx = np.sort(cidx[keep])
            K = len(cidx)
            lo = int(SLOT_BASE[s])
            # lhs rows: per coord d -> [ah_d, ah_d, al_d]; rows 9,10 = -1
            w = wg[b]                                         # [100, 3]
            ah, al = _f16_split(w.T / 4.0)                    # [3, 100]
            for d in range(3):
                lhsc[3 * d + 0, s * 128:s * 128 + T] = ah[d]
                lhsc[3 * d + 1, s * 128:s * 128 + T] = ah[d]
                lhsc[3 * d + 2, s * 128:s * 128 + T] = al[d]
            lhsc[9, s * 128:(s + 1) * 128] = np.float16(-1.0)
            lhsc[10, s * 128:(s + 1) * 128] = np.float16(-1.0)
            # rhs rows: per coord d -> [bh_d, bl_d, bh_d]; then [ch, cl]
            for d in range(3):
                rhsc[3 * d + 0, lo:lo + K] = bh[d, cidx]
                rhsc[3 * d + 1, lo:lo + K] = bl[d, cidx]
                rhsc[3 * d + 2, lo:lo + K] = bh[d, cidx]
            rhsc[9, lo:lo + K] = ch[cidx]
            rhsc[10, lo:lo + K] = cl[cidx]
            # merged gather table: row p = [data(p), data(p+half)];
            # pad rows get p2 = 1e30 so a padded candidate can never win
            gtabc = np.zeros((cap, 8), np.float32)
            gtabc[:K] = gflat[cidx]
            gtabc[K:, 3] = 1.0e30
            half = cap // 2
            imap[f"gt{s}"] = np.concatenate(
                [gtabc[:half], gtabc[half:]], axis=1).copy()
            wgmc[:T, s * 3:(s + 1) * 3] = w
        imap["wgm"] = wgmc
        in_maps.append(imap)
    return in_maps


_CACHE = {}


def kernel(posesglobal, waypointslocal, boundary, boundarynormals):
    if "nc" not in _CACHE:
        _CACHE["nc"] = build()
    nc = _CACHE["nc"]
    in_maps = prep_inputs(posesglobal, waypointslocal, boundary,
                          boundarynormals)
    res = run_bass_kernel_spmd(nc, in_maps, list(range(NCORES)))
    total = 0.0
    for r in res.results:
        total += float(np.asarray(r["out"], dtype=np.float64).sum())
    return np.float32(total / (B * T))
